# revision 20
# baseline (speedup 1.0000x reference)
# Trainium2 Bass kernel for nn_EquivariantLocalScoreMachine (retrieval_knn).
#
# Math: for each spatial site s=(b,y,x) (S=2048) and dataset patch p (P=32768):
#   w[p,s] = (mu*conv[p,s] - (x_norms[s] + mu^2*pnorms[p])/2) / sigma^2
#   out[c,s] = (mu * sum_p e^w*pcent[p,c] / sum_p e^w - x[c,s]) / sigma^2
# The output is invariant to any per-site offset of w; a host-side
# Cauchy-Schwarz bound M~[s] (slack measured 0.9..2.4 on this data) is folded
# into the matmul so weights peak near e^5.8, inside fp8e4m3 range.
#
# Device kernel (per core, patches sharded 8 ways -> 4096 patches/core).
# Three engine-level tricks vs the naive (ACT-only exp, fp16 serial matmuls):
#   1. exp SPLIT across ACT and DVE working in parallel out of PSUM. The
#      matmul emits y = C1*w + C2 where (C1,C2) are the fp8e4m3 Schraudolph
#      constants: ACT computes exact exp via its free affine
#      (exp(y/C1 - C2/C1) -> f8e4 values); DVE does one tensor_scalar_max
#      (fp32->int8, round-half-even, verified) whose bits ARE ~e^w in e4m3
#      (~5% zero-mean noise; harmless: weight Neff ~ 31000, tol 2e-2).
#   2. w-matmuls use a 58-row fp16 [Xh;Xl]x[Ph;Ph] stack, replicated at
#      base partitions 0 and 64: the two matmuls of a patch-tile pair go to
#      row-groups (0,0)/(64,0) and overlap on the 16x(32x32) PE array.
#   3. PV-matmuls run fp8 DoubleRow: one MM contracts a 256-patch pair
#      (lhsT [128,2,16] zero-padded pc, rhs [128,2,512] wexp bits) -> half
#      the PE streaming of the fp16 version. DoubleRow cannot column-tile,
#      so the 4 site-tile accumulators stack at partitions 4j of one
#      [16,512] R bank via per-j zero-padded lhsT columns.
# PSUM ring: 3 window tensors x 2 banks (one patch-pair x one 512-site tile)
# + 1 bank of PV accumulators R; windows are assigned to ACT/DVE by a greedy
# balance plan; PV trails by skew_w windows.
# Host combines the 8 cores' partial sums (offset cancels in the ratio).

import os
import sys

for _p in ("/opt/trn_rl_repo", "/root/.axon_site/_ro/trn_rl_repo"):
    if os.path.isdir(_p) and _p not in sys.path:
        sys.path.insert(0, _p)

import numpy as np

N_CORES = 8

# fp8e4m3 Schraudolph constants: y = C1*w + C2; int8(y) bits ~ e^w.
# SIG zero-means the mantissa-linear approximation error.
_SIG = 0.0576
_C1 = 8.0 / np.log(2.0)
_C2 = 8.0 * (7.0 - _SIG)
_SHIFT = 5.8               # weights peak near e^SHIFT (fp8e4m3 max 448)

_PROGRAM_CACHE = {}


def _split16(v):
    hi = v.astype(np.float16)
    lo = (v - hi.astype(np.float32)).astype(np.float16)
    return hi, lo


def _split8(v):
    import ml_dtypes
    hi = v.astype(ml_dtypes.float8_e4m3)
    lo = (v - hi.astype(np.float32)).astype(ml_dtypes.float8_e4m3)
    return hi, lo


def _preprocess(x, images, mu, sigma, t, layout="2way58"):
    x = np.ascontiguousarray(np.asarray(x, np.float32))
    images = np.asarray(images, np.float32)
    t = int(np.asarray(t))
    mu_t = float(np.asarray(mu)[t])
    sig_t = float(np.asarray(sigma)[t])
    s2 = sig_t * sig_t
    a = mu_t / s2
    bconst = mu_t * mu_t / (2.0 * s2)
    B, C, H, W = x.shape
    imgs = images.reshape(-1, C, H, W)
    N_all = imgs.shape[0]
    S = B * H * W
    K = 3

    # dataset patches [P, 27], flatten order (c, di, dj); zero padding
    pz = np.pad(imgs, ((0, 0), (0, 0), (1, 1), (1, 1)))
    wins = np.empty((N_all, C, K, K, H, W), np.float32)
    for di in range(K):
        for dj in range(K):
            wins[:, :, di, dj] = pz[:, :, di:di + H, dj:dj + W]
    patches = wins.transpose(0, 4, 5, 1, 2, 3).reshape(N_all * H * W, C * K * K)
    P = patches.shape[0]
    pcent = patches[:, [4, 13, 22]]  # (c, di=1, dj=1) -> c*9+4
    pnorms = (patches.astype(np.float64) ** 2).sum(1).astype(np.float32)

    # x-side windows [S, 27], s = (b, y, x); circular padding
    xp = np.pad(x, ((0, 0), (0, 0), (1, 1), (1, 1)), mode="wrap")
    xwins = np.empty((B, C, K, K, H, W), np.float32)
    for di in range(K):
        for dj in range(K):
            xwins[:, :, di, dj] = xp[:, :, di:di + H, dj:dj + W]
    Xw = xwins.transpose(0, 4, 5, 1, 2, 3).reshape(S, C * K * K)
    x_norms = Xw.sum(1) ** 2
    xn2 = np.sqrt((Xw.astype(np.float64) ** 2).sum(1)).astype(np.float32)

    # per-site upper bound on max_p w, shifted so wexp peaks near e^SHIFT
    Mt = (abs(a) * xn2 * np.sqrt(pnorms.max()) - x_norms / (2 * s2)
          - bconst * pnorms.min() - _SHIFT).astype(np.float32)

    # matmul emits y = C1*w' + C2 (w' = w - Mt): scale the x side by C1 and
    # fold C2 into the site-constant row.
    Xmat = np.empty((29, S), np.float32)
    Xmat[0:27] = Xw.T * (a * _C1)
    Xmat[27] = _C1
    Xmat[28] = _C1 * (-x_norms / (2 * s2) - Mt) + _C2
    Pmat = np.empty((29, P), np.float32)
    Pmat[0:27] = patches.T
    Pmat[27] = -bconst * pnorms
    Pmat[28] = 1.0

    # "2way58": 58-row fp16 [Xh;Xl]x[Ph;Ph] stack at partitions 0 and 64
    # (patch-tile pairs row-tile the PE array 2-way). "1stack": single
    # 29-row fp16 stack at 0/32/64/96 (for 2/3/4-way experiments).
    xmat2 = np.zeros((125, S), np.float16)
    pmat2 = np.zeros((125, P), np.float16)
    if layout == "2way58":
        Xh, Xl = _split16(Xmat)
        Ph = Pmat.astype(np.float16)
        xstack = np.concatenate([Xh, Xl], 0)                   # [58, S]
        pstack = np.concatenate([Ph, Ph], 0)                   # [58, P]
        xmat2[0:58] = xstack
        xmat2[64:122] = xstack
        pmat2[0:58] = pstack
        pmat2[64:122] = pstack
    else:
        xstack = Xmat.astype(np.float16)
        pstack = Pmat.astype(np.float16)
        for r in range(4):
            xmat2[32 * r:32 * r + 29] = xstack
            pmat2[32 * r:32 * r + 29] = pstack

    # pc in fp8 (hi only: costs ~4e-5 output error) + ones. DoubleRow cannot
    # column-tile, so the PV lhsT is zero-padded per site-tile j: values live
    # at columns 4j..4j+3 of a 16-wide (k-step %16) block and every PV
    # writes the full [16,512] R.
    import ml_dtypes
    pch = pcent.astype(ml_dtypes.float8_e4m3)
    pc_aug = np.zeros((P, 4, 16), ml_dtypes.float8_e4m3)
    for j in range(4):
        pc_aug[:, j, 4 * j + 0:4 * j + 3] = pch
        pc_aug[:, j, 4 * j + 3] = 1.0

    return dict(xmat2=xmat2, pmat2=pmat2, pc_aug=pc_aug,
                mu_t=mu_t, s2=s2, x=x, B=B, C=C, H=H, W=W, S=S, P=P)


def _make_window_plan(n_win, ca, cd, mode="greedy"):
    """ACT/DVE assignment per window. 'greedy' balances busy time; 'alt'
    strictly alternates; 'alt+N' alternates with an extra A every N."""
    if mode == "alt":
        return ["A" if i % 2 == 0 else "D" for i in range(n_win)]
    if mode.startswith("alt+"):
        n = int(mode[4:])
        plan = []
        k = 0
        for i in range(n_win):
            if i % n == n - 1:
                plan.append("A")
            else:
                plan.append("A" if k % 2 == 0 else "D")
                k += 1
        return plan
    plan = []
    t_act = t_dve = 0.0
    for _ in range(n_win):
        if t_act + ca <= t_dve + cd:
            plan.append("A")
            t_act += ca
        else:
            plan.append("D")
            t_dve += cd
    # the loop barrier waits for the LAST window's exp: end on the cheaper
    # ACT op (swap keeps the engine balance intact)
    if plan[-1] == "D":
        for i in range(n_win - 2, -1, -1):
            if plan[i] == "A":
                plan[i], plan[-1] = plan[-1], plan[i]
                break
    return plan


def _build_program_v5(S, P_core, repeat=1, loop_n=None, skew_w=2, we_bufs=4,
                      stag=False, all_act=False, plan_mode="greedy",
                      pv_block=False, no_rowtile=False, rowtile="2way58",
                      all_dve=False, plan_ca=1073.0, plan_cd=1427.0):
    import contextlib

    import concourse.bacc as bacc
    import concourse.mybir as mybir
    import concourse.tile as tile

    f16 = mybir.dt.float16
    f32 = mybir.dt.float32
    f8 = mybir.dt.float8e4
    i8 = mybir.dt.int8
    NT = P_core // 128          # 32 patch-tiles
    NT2 = NT // 2               # 16 patch-tile pairs
    NS = S // 512               # 4 site-tiles
    NW = NT2 * NS               # 64 windows per iteration
    assert NS == 4 and NT % 8 == 0

    nc = bacc.Bacc("TRN2", target_bir_lowering=False, debug=False,
                   num_devices=N_CORES)
    xmat_d = nc.declare_dram_parameter("xmat", (125, S), f16, isOutput=False)
    pmats_d = nc.declare_dram_parameter("pmats", (125, P_core), f16,
                                        isOutput=False)
    pcents_d = nc.declare_dram_parameter("pcents", (128, NS, NT2, 2, 16), f8,
                                         isOutput=False)
    rout_d = nc.declare_dram_parameter("r_out", (NS * 4, 512), f32,
                                       isOutput=True)

    # engine costs per [128,1024] window, HW-calibrated (all-ACT / all-DVE
    # runs measured 68.7us and 91.3us over 64 windows)
    plan = _make_window_plan(NW, plan_ca, plan_cd, plan_mode)
    if all_act:
        plan = ["A"] * NW
    if all_dve:
        plan = ["D"] * NW

    with tile.TileContext(nc) as tc:
        with tc.tile_pool(name="const", bufs=1) as const, \
             tc.tile_pool(name="wexp", bufs=we_bufs) as wpool, \
             tc.tile_pool(name="psw", bufs=1, space="PSUM") as psw, \
             tc.tile_pool(name="psr", bufs=1, space="PSUM") as psr:

            # warm the exp table while DMAs stream
            dummy = const.tile([128, 1], f32, tag="dummy")
            nc.vector.memset(dummy[:], 0.0)
            nc.scalar.activation(dummy[:], dummy[:],
                                 mybir.ActivationFunctionType.Exp)

            bias_t = const.tile([128, 1], f32, tag="bias")
            nc.vector.memset(bias_t[:], -float(_C2 / _C1))

            xmat_t = const.tile([125, S], f16, tag="xmat")
            for q in range(4):
                nc.sync.dma_start(out=xmat_t[:, q * (S // 4):(q + 1) * (S // 4)],
                                  in_=xmat_d[:, q * (S // 4):(q + 1) * (S // 4)])
            pc_t = const.tile([128, NS, NT2, 2, 16], f8, tag="pc")
            nc.sync.dma_start(out=pc_t[:], in_=pcents_d[:])
            pm_t = []
            chunk = NT // 4 * 128
            for q in range(4):
                pt = const.tile([125, chunk], f16, tag=f"pm{q}", name=f"pm{q}")
                nc.sync.dma_start(out=pt[:],
                                  in_=pmats_d[:, q * chunk:(q + 1) * chunk])
                pm_t.append(pt)

            # PSUM: 3 window tensors x 2 banks + 1 bank R = 7 of 8 banks.
            R = psr.tile([16, 512], f32, tag="R")
            wt_t = [psw.tile([128, 1024], f32, tag=f"wt{k}", name=f"wt{k}")
                    for k in range(3)]

            loop_cm = (tc.For_i(0, loop_n, 1,
                                hint_engines=(mybir.EngineType.PE,
                                              mybir.EngineType.Activation,
                                              mybir.EngineType.DVE),
                                staggered_reset=stag)
                       if loop_n else contextlib.nullcontext())
            with loop_cm:
                for rep in range(repeat if not loop_n else 1):
                    pending = []

                    def emit_pv(ent):
                        wi, q, j, we = ent
                        nc.tensor.matmul(
                            R[:],
                            pc_t[:, j, q, :, 0:16],
                            we[:].bitcast(f8),
                            start=(wi == 0), stop=(wi == NW - 1),
                            perf_mode=mybir.MatmulPerfMode.DoubleRow,
                            skip_group_check=True,
                            tile_position=(0, 0))

                    for wi in range(NW):
                        j, q = wi // NT2, wi % NT2
                        wt = wt_t[wi % 3]
                        for k in range(2):
                            i = 2 * q + k
                            lhs = pm_t[i // (NT // 4)]
                            ci = (i % (NT // 4)) * 128
                            if rowtile == "3way29":
                                rb = 32 * (i % 3)
                                nr = 29
                            elif rowtile == "2way29":
                                rb = 64 * (i % 2)
                                nr = 29
                            elif rowtile == "4way29":
                                rb = 32 * (i % 4)
                                nr = 29
                            else:
                                rb = 0 if no_rowtile else 64 * (i % 2)
                                nr = 58
                            nc.tensor.matmul(
                                wt[:, 512 * k:512 * (k + 1)],
                                lhs[rb:rb + nr, ci:ci + 128],
                                xmat_t[rb:rb + nr, 512 * j:512 * (j + 1)],
                                start=True, stop=True,
                                tile_position=(rb, 0))
                        we = wpool.tile([128, 2, 512], i8, tag=f"we{wi % 3}",
                                        name=f"we{wi % 3}")
                        if plan[wi] == "A":
                            nc.scalar.activation(
                                we[:].bitcast(f8), wt[:],
                                mybir.ActivationFunctionType.Exp,
                                bias=bias_t[:], scale=float(1.0 / _C1))
                        else:
                            nc.vector.tensor_scalar_max(we[:], wt[:], 0.0)
                        pending.append((wi, q, j, we))
                        if pv_block:
                            if q == NT2 - 1:
                                for ent in pending:
                                    emit_pv(ent)
                                pending = []
                        elif len(pending) > skew_w:
                            emit_pv(pending.pop(0))
                    for ent in pending:
                        emit_pv(ent)
            r_sb = const.tile([16, 512], f32, tag="r_sb")
            nc.vector.tensor_copy(r_sb[:], R[:])
            nc.sync.dma_start(out=rout_d[:], in_=r_sb[:])
    nc.compile()
    return nc


def _get_program_best(S, P_core, loop_n=None):
    key = ("best", S, P_core, loop_n)
    if key not in _PROGRAM_CACHE:
        _PROGRAM_CACHE[key] = _build_program_v5(S, P_core, loop_n=loop_n,
                                                skew_w=4, we_bufs=6,
                                                plan_mode="greedy",
                                                plan_cd=1550.0, stag=True)
    return _PROGRAM_CACHE[key]


def _make_in_maps(d):
    P_core = d["P"] // N_CORES
    NT = P_core // 128
    NT2 = NT // 2
    in_maps = []
    for c in range(N_CORES):
        sl = slice(c * P_core, (c + 1) * P_core)
        pc_block = d["pc_aug"][sl].reshape(NT2, 2, 128, 4, 16)
        pc_core = np.ascontiguousarray(pc_block.transpose(2, 3, 0, 1, 4))
        in_maps.append({
            "xmat": d["xmat2"],
            "pmats": np.ascontiguousarray(d["pmat2"][:, sl]),
            "pcents": pc_core,
        })
    return in_maps


def _postprocess(d, results):
    S, C, B, H, W = d["S"], d["C"], d["B"], d["H"], d["W"]
    R = np.zeros((16, 512), np.float64)
    for c in range(N_CORES):
        R += results[c]["r_out"].astype(np.float64)
    R = R.reshape(4, 4, 512)
    Rc = R[:, 0:3, :].transpose(1, 0, 2).reshape(C, S)
    sw = R[:, 3, :].reshape(S)
    xs = d["x"].transpose(1, 0, 2, 3).reshape(C, S)
    out = (d["mu_t"] * Rc / sw - xs) / d["s2"]
    return np.ascontiguousarray(
        out.reshape(C, B, H, W).transpose(1, 0, 2, 3)).astype(np.float32)


def kernel(x, images, mu, sigma, t):
    from concourse.bass_utils import run_bass_kernel_spmd

    d = _preprocess(x, images, mu, sigma, t)
    assert d["P"] % (N_CORES * 256) == 0
    nc = _get_program_best(d["S"], d["P"] // N_CORES)
    res = run_bass_kernel_spmd(nc, _make_in_maps(d), list(range(N_CORES)))
    return _postprocess(d, res.results)
